# revision 2
# baseline (speedup 1.0000x reference)
"""ConvSP (SPH message-passing conv) Trainium2 kernel — ap_gather version.

Problem (per full input): B=2 batches, N=8192 particles, M=8192 queries,
K=32 neighbors, C=16 in channels, O=16 out channels, 27 kernel cells.

    out[b,m,o] = bias[o] + sum_{e,k,c} kv(b,m,e,k) * data[b, nbr[b,m,k], c] * W[o,c,e]
    kv = relu(1 - sqrt(|qloc + off_e - loc_nbr|^2 + 1e-12)/R)^3

Sharding: 8 cores = 2 batches x 4 query-quarters (2048 queries each), SPMD.

Gather strategy (the old dma_gather cost ~62us per 8192 refs because one Q7
core pair generates every descriptor): use ap_gather instead. The fp16
record table [p, n, 2] holds field-pair p%16 of particle n (10 pairs:
(lx,ly),(lz,0),(d0,d1)..(d14,d15)), replicated into all 8 gpsimd core
groups. Each core group gathers ITS OWN 1024-ref slice of the 8192-ref
subtile stream, so all 8 Q7 cores work in parallel.

Stream layout: core group g=(2h+hf) covers (t=h, k=c%32, j=hf*32+c//32).
After a DVE 32x32 block transpose of the [128, 1024] f32 (fp16-pair) gather
output, partition (32t+k) column-group cb holds the records of chunks
j=cb (words 0..9) and j=cb+32 (words 16..25) — i.e. the classic
[(t,k), j, fields] layout the compute pipeline wants.

Downstream per chunk (4 queries x 32 neighbors on 128 partitions) is the
same structure as the dma_gather kernel: separable cell-offset DVE math for
kv, block-diag slotted data + per-chunk PE matmul for acc, 27 accumulated
PE matmuls against block-diag W for the output, all in fp16 where harmless.
"""
import os
import sys
import numpy as np
from contextlib import ExitStack

sys.path.insert(0, "/opt/trn_rl_repo")

import concourse.bass as bass
import concourse.bacc as bacc
import concourse.mybir as mybir
import concourse.tile as tile
from concourse.masks import make_identity
from concourse.bass_utils import run_bass_kernel_spmd

F32 = mybir.dt.float32
F16 = mybir.dt.float16
I32 = mybir.dt.int32
I16 = mybir.dt.int16
AF = mybir.ActivationFunctionType

P = 128          # partitions
NQ = 2048        # queries per core
N = 8192         # particles per batch
K = 32           # neighbors
C = 16           # in channels
O = 16           # out channels
D = 3
E = 27           # cells
T = 4            # queries per chunk
J = NQ // T      # chunks per core = 512
JS = 64          # chunks per subtile
NSUB = J // JS   # 8
NI = JS * 16     # gathered refs per core group per subtile = 1024
RADIUS = 0.1
DIL = 0.05


def build_program():
    nc = bacc.Bacc("TRN2", target_bir_lowering=False, debug=False,
                   num_devices=8)

    tab_d = nc.declare_dram_parameter("tab", [P, N * 2], F16, isOutput=False)
    idx_d = nc.declare_dram_parameter("idx", [P, NSUB * JS], I16, isOutput=False)
    qt3_d = nc.declare_dram_parameter("qt3", [P, J * D], F32, isOutput=False)
    wbd_d = nc.declare_dram_parameter("wbd", [64, E * 64], F16, isOutput=False)
    bias4_d = nc.declare_dram_parameter("bias4", [64], F32, isOutput=False)
    out_d = nc.declare_dram_parameter("out", [NQ, O], F32, isOutput=True)

    with tile.TileContext(nc) as tc:
        with ExitStack() as ctx:
            _build(ctx, tc, tab_d, idx_d, qt3_d, wbd_d, bias4_d, out_d)
    nc.finalize()
    return nc


def _build(ctx, tc, tab_d, idx_d, qt3_d, wbd_d, bias4_d, out_d):
    nc = tc.nc

    consts = ctx.enter_context(tc.tile_pool(name="consts", bufs=1))
    agp = ctx.enter_context(tc.tile_pool(name="agp", bufs=2))
    trp = ctx.enter_context(tc.tile_pool(name="trp", bufs=2))
    dve = ctx.enter_context(tc.tile_pool(name="dve", bufs=2))
    kvp = ctx.enter_context(tc.tile_pool(name="kv", bufs=2))
    sbdp = ctx.enter_context(tc.tile_pool(name="sbdp", bufs=2))
    accs = ctx.enter_context(tc.tile_pool(name="accs", bufs=1))
    outs = ctx.enter_context(tc.tile_pool(name="outs", bufs=2))
    accps = ctx.enter_context(tc.tile_pool(name="accps", bufs=4, space="PSUM"))
    outps = ctx.enter_context(tc.tile_pool(name="outps", bufs=2, space="PSUM"))
    trps = ctx.enter_context(tc.tile_pool(name="trps", bufs=2, space="PSUM"))

    # ---------------- constants ----------------
    oxc = consts.tile([P, D], F32)      # 2*off(e)
    ox2c = consts.tile([P, D], F32)     # off(e)^2
    for i in range(D):
        off = (i - 1) * DIL
        nc.vector.memset(oxc[:, i:i + 1], 2.0 * off)
        nc.vector.memset(ox2c[:, i:i + 1], off * off)
    epsb = consts.tile([P, 1], F32)
    nc.vector.memset(epsb[:], 1e-12)
    oneb = consts.tile([P, 1], F32)
    nc.vector.memset(oneb[:], 1.0)
    ident = consts.tile([64, 64], F32)
    make_identity(nc, ident[:])
    # table + index first (they gate gather 0); table split across both
    # HWDGE rings — a single 4MB DMA crawls at ~90GB/s on one ring.
    idxs = consts.tile([P, NSUB * JS], I16)
    nc.sync.dma_start(idxs[:], idx_d[:])
    tab = consts.tile([P, N * 2], F16)
    nc.sync.dma_start(tab[:, :N], tab_d[:, :N])
    nc.scalar.dma_start(tab[:, N:], tab_d[:, N:])
    bias4 = consts.tile([64, 1], F32)
    nc.sync.dma_start(bias4[:], bias4_d[:].rearrange("(p o) -> p o", o=1))
    wbd = consts.tile([64, E * 64], F16)
    nc.scalar.dma_start(wbd[:], wbd_d[:])
    qt3 = consts.tile([P, J * D], F32)
    nc.scalar.dma_start(qt3[:], qt3_d[:])

    # ---------------- per-subtile buffers ----------------
    # one gather call covers TWO subtiles (2048 idxs/core) to amortize the
    # ~5us engine round-trip per ap_gather
    ags = [agp.tile([P, NI * 2], F16, tag=f"ag{i}", name=f"ag{i}")
           for i in range(2)]
    trs = [trp.tile([P, NI], F32, tag=f"tr{i}", name=f"tr{i}")
           for i in range(2)]
    sbds = [sbdp.tile([P, JS * 64], F16, tag=f"sb{i}", name=f"sb{i}")
            for i in range(2)]
    nc.vector.memset(sbds[0][:], 0.0)
    nc.vector.memset(sbds[1][:], 0.0)
    accbig = accs.tile([64, 4 * JS * E], F16, tag="accbig", name="accbig")
    acc4 = [accbig[:, i * JS * E:(i + 1) * JS * E] for i in range(4)]

    def gather_block(s):
        nc.gpsimd.ap_gather(
            ags[s % 2][:],
            tab[:],
            idxs[:, s * JS:(s + 1) * JS],
            channels=P, num_elems=N, d=2, num_idxs=NI)

    def subtile(s):
        ag = ags[s % 2]
        tr = trs[s % 2]
        sbd = sbds[s % 2]
        # 32x32 block transpose: tr[32t+k, 32cb+w] = ag32[32t+w, 32cb+k]
        nc.vector.transpose(tr[:], ag[:].bitcast(F32))
        # fp16 view: trh[p, cb, hf*32 + f], f: 0..2 loc xyz, 4+c data
        trh = tr[:].bitcast(F16).rearrange("p (cb z) -> p cb z", z=64)

        # --- block-diag data: copy c-fields into slot t (fp16)
        sv = sbd[:].rearrange("p (j s c) -> p j s c", s=T, c=C)
        for t in range(T):
            for hf in range(2):
                src = trh[t * K:(t + 1) * K, :, hf * 32 + 4:hf * 32 + 20]
                dst = sv[t * K:(t + 1) * K, hf * K:hf * K + K, t, :]
                if t < 2:
                    nc.vector.tensor_copy(dst, src)
                else:
                    nc.scalar.activation(dst, src, AF.Copy)

        # --- nloc (f32) from fp16 pairs; delta = q - l
        nl3 = dve.tile([P, JS * D], F32, tag="nl3")
        nl3v = nl3[:].rearrange("p (hf cb d) -> p hf cb d", hf=2, d=D)
        for hf in range(2):
            nc.vector.tensor_copy(nl3v[:, hf, :, :], trh[:, :, hf * 32:hf * 32 + D])
        d3 = dve.tile([P, JS * D], F32, tag="d3")
        q_v = qt3[:].rearrange("p (j d) -> p j d", d=D)
        nc.vector.tensor_sub(
            d3[:].rearrange("p (hf cb d) -> p (hf cb) d", hf=2, d=D),
            q_v[:, s * JS:(s + 1) * JS, :],
            nl3[:].rearrange("p (hf cb d) -> p (hf cb) d", hf=2, d=D))

        d3v = d3[:].rearrange("p (j d) -> p j d", d=D)
        dx, dy, dz = d3v[:, :, 0], d3v[:, :, 1], d3v[:, :, 2]

        # --- s2 = dx^2+dy^2+dz^2
        s2 = dve.tile([P, JS], F32, tag="s2")
        tmp = dve.tile([P, JS], F32, tag="tmp")
        nc.vector.tensor_mul(s2[:], dx, dx)
        nc.vector.tensor_mul(tmp[:], dy, dy)
        nc.vector.tensor_add(s2[:], s2[:], tmp[:])
        nc.vector.tensor_mul(tmp[:], dz, dz)
        nc.vector.tensor_add(s2[:], s2[:], tmp[:])

        # --- per-axis terms p*[j,e] = 2*off*d + off^2 (+ s2 on x)
        def axis_term(dcomp, add_s2, tg):
            pt = dve.tile([P, JS * D], F32, tag=tg)
            ptv = pt[:].rearrange("p (j e) -> p j e", e=D)
            din = dcomp.unsqueeze(2).broadcast_to((P, JS, D))
            oc = oxc[:].unsqueeze(1).broadcast_to((P, JS, D))
            o2 = ox2c[:].unsqueeze(1).broadcast_to((P, JS, D))
            nc.vector.tensor_mul(ptv, din, oc)
            nc.vector.tensor_add(ptv, ptv, o2)
            if add_s2:
                s2b = s2[:].unsqueeze(2).broadcast_to((P, JS, D))
                nc.vector.tensor_add(ptv, ptv, s2b)
            return pt

        pxe = axis_term(dx, True, "pxe")
        pye = axis_term(dy, False, "pye")
        pze = axis_term(dz, False, "pze")

        # --- u2[j,ex,ey] = pxe+pye ; d2[j,ex,ey,ez] = u2+pze
        u2 = dve.tile([P, JS * 9], F32, tag="u2")
        u2v = u2[:].rearrange("p (j a b) -> p j a b", a=D, b=D)
        nc.vector.tensor_add(
            u2v,
            pxe[:].rearrange("p (j a) -> p j a", a=D).unsqueeze(3)
                  .broadcast_to((P, JS, D, D)),
            pye[:].rearrange("p (j b) -> p j b", b=D).unsqueeze(2)
                  .broadcast_to((P, JS, D, D)))
        d2 = kvp.tile([P, JS * E], F32, tag="d2")
        d2v = d2[:].rearrange("p (j a b) -> p j a b", a=9, b=D)
        nc.vector.tensor_add(
            d2v,
            u2[:].rearrange("p (j a) -> p j a", a=9).unsqueeze(3)
                 .broadcast_to((P, JS, 9, D)),
            pze[:].rearrange("p (j b) -> p j b", b=D).unsqueeze(2)
                  .broadcast_to((P, JS, 9, D)))

        # --- kv = relu(1 - sqrt(d2+eps)/R)^3, fp16 out for the PE matmul
        nc.scalar.activation(d2[:], d2[:], AF.Sqrt, bias=epsb[:])
        nc.scalar.activation(d2[:], d2[:], AF.Relu, bias=oneb[:],
                             scale=-1.0 / RADIUS)
        sq = kvp.tile([P, JS * E], F32, tag="sq")
        nc.vector.tensor_mul(sq[:], d2[:], d2[:])
        kv = kvp.tile([P, JS * E], F16, tag="kvt")
        nc.vector.tensor_mul(kv[:], sq[:], d2[:])

        # --- acc[(t,c), e] per chunk on PE
        kvv = kv[:].rearrange("p (j e) -> p j e", e=E)
        acc_sb = acc4[s % 4]
        for grp in range(JS // 16):
            ap_ps = accps.tile([64, 16 * E], F32, tag="accps")
            for jl in range(16):
                jj = grp * 16 + jl
                nc.tensor.matmul(ap_ps[:, jl * E:(jl + 1) * E],
                                 sbd[:, jj * 64:(jj + 1) * 64],
                                 kvv[:, jj, :],
                                 start=True, stop=True)
            nc.scalar.activation(acc_sb[:, grp * 16 * E:(grp + 1) * 16 * E],
                                 ap_ps[:], AF.Copy)

    def final_group(fg):
        # --- out[(t,o), jtot] = sum_e Wbd_e @ acc_e over 4 subtiles (256 cols)
        JT = 4 * JS
        op = outps.tile([64, JT], F32, tag="outps")
        accv = accbig[:].rearrange("p (jt e) -> p jt e", e=E)
        for e in range(E):
            nc.tensor.matmul(op[:], wbd[:, e * 64:(e + 1) * 64],
                             accv[:, :, e],
                             start=(e == 0), stop=(e == E - 1))
        osb = outs.tile([64, JT], F32, tag="osb")
        nc.scalar.activation(osb[:], op[:], AF.Identity, bias=bias4[:])

        # --- transpose to [j, (t,o)] and store contiguously
        out_v = out_d[:].rearrange("(s j t) o -> s j (t o)", s=NSUB, t=T)
        for q in range(4):
            trq = trps.tile([64, 64], F32, tag="trq")
            nc.tensor.transpose(trq[:], osb[:, q * 64:(q + 1) * 64], ident[:])
            trsb = outs.tile([64, 64], F32, tag="trsb")
            nc.scalar.activation(trsb[:], trq[:], AF.Copy)
            nc.sync.dma_start(out_v[fg * 4 + q], trsb[:])

    # ---------------- pipeline ----------------
    gather_block(0)
    for s in range(NSUB):
        if s + 1 < NSUB:
            gather_block(s + 1)
        subtile(s)
        if s % 4 == 3:
            final_group(s // 4)


_PROGRAM = None


def _get_program():
    global _PROGRAM
    if _PROGRAM is None:
        _PROGRAM = build_program()
    return _PROGRAM


def _prep_core(qlocs_b, nbrs_q, locs_b, data_b):
    """Host-side layout prep for one core (batch b, query quarter qq)."""
    # record table: fp16 field-pairs, replicated into 8 gpsimd core groups
    rec = np.zeros((16, N, 2), np.float16)
    rec[0, :, 0] = locs_b[:, 0]
    rec[0, :, 1] = locs_b[:, 1]
    rec[1, :, 0] = locs_b[:, 2]
    rec[2:10] = data_b.astype(np.float16).reshape(N, 8, 2).transpose(1, 0, 2)
    tab = np.broadcast_to(rec.reshape(1, 16, N * 2), (8, 16, N * 2))
    tab = np.ascontiguousarray(tab).reshape(P, N * 2)

    # index streams: core group g=(2t+hf): c -> (t, k=c%32, j=hf*32+c//32)
    idx = np.zeros((P, NSUB * JS), np.int16)
    for g in range(8):
        t, hf = g // 2, g % 2
        for s in range(NSUB):
            jg = s * JS + hf * K + np.arange(K)          # [jj]
            m = 4 * jg + t
            vals = nbrs_q[m, :]                          # [jj, k] -> c = jj*32+k
            st = vals.reshape(-1).astype(np.int16)
            idx[16 * g:16 * g + 16, s * JS:(s + 1) * JS] = \
                st.reshape(JS, 16).T
    # qt3[t*32+k, jg, d] = qlocs[4*jg+t, d]
    q4 = qlocs_b.reshape(J, T, D).transpose(1, 0, 2)     # [t, jg, d]
    qt3 = np.broadcast_to(q4[:, None], (T, K, J, D))
    qt3 = np.ascontiguousarray(qt3).reshape(P, J * D)
    return tab, idx, qt3


def kernel(qlocs, locs, data, neighbors, weight, bias):
    B, M = qlocs.shape[0], qlocs.shape[1]
    assert (B, M) == (2, 8192)
    ncores = 8

    wbd = np.zeros((E, 64, 64), np.float32)
    w = np.asarray(weight, np.float32)           # [O, C, E]
    for t in range(T):
        wbd[:, t * C:(t + 1) * C, t * O:(t + 1) * O] = w.transpose(2, 1, 0)
    wbd = np.ascontiguousarray(
        wbd.transpose(1, 0, 2).reshape(64, E * 64)).astype(np.float16)
    bias4 = np.tile(np.asarray(bias, np.float32), T)

    in_maps = []
    for cid in range(ncores):
        b, qq = cid // 4, cid % 4
        sl = slice(qq * NQ, (qq + 1) * NQ)
        tab, idx, qt3 = _prep_core(
            np.asarray(qlocs[b, sl], np.float32),
            np.asarray(neighbors[b, sl], np.int32),
            np.asarray(locs[b], np.float32),
            np.asarray(data[b], np.float32))
        in_maps.append({
            "tab": tab, "idx": idx, "qt3": qt3,
            "wbd": wbd, "bias4": bias4,
        })

    nc = _get_program()
    res = run_bass_kernel_spmd(nc, in_maps, list(range(ncores)),
                               trace=bool(int(os.environ.get("CONVSP_TRACE", "0"))))
    out = np.zeros((B, M, O), np.float32)
    for cid in range(ncores):
        b, qq = cid // 4, cid % 4
        out[b, qq * NQ:(qq + 1) * NQ] = res.results[cid]["out"]
    kernel.last_results = res
    return out
